# revision 1
# baseline (speedup 1.0000x reference)
"""Trainium2 Bass kernel for nn_CrossAttentionAdapter.

Math note: the reference's attention has kv_len == 1, so the softmax over a
length-1 axis is exactly 1.0 and the attention output is just `v` broadcast
over the P=32 prefix positions.  The whole module therefore collapses to a
chain of 4 matmuls applied to image_embs:

    row = image_embs @ Wm.T @ Wv.T @ Wo_mha.T @ Wo.T  (+ bias constant)
    out[b, p, :] = row[b, :]          for every p in range(32)

where Wv = Win[2E:3E].  The bias terms contribute a batch-independent
constant vector c = ((bm @ Wv.T + bv) @ Wo_mha.T + bo_mha) @ Wo.T + bo,
added on the host (it is a few matvecs).  prefix_queries / Wq / Wk never
affect the output.

Device strategy (pure data parallel, 8 cores):
  - batch (1024) sharded 8-ways -> 128 rows per core
  - weights replicated, cast to bf16, pre-transposed on the host
  - per core: 4-layer matmul chain; activations stay transposed (feature
    dim on partitions) the whole way, used as the moving operand; weight
    tiles are the stationary operand (bf16 fast-weight-load)
  - the 16 output-tile accumulators of a layer are packed 4-per-PSUM-bank
    as [128,512] tiles, so a full layer accumulates with only 4 banks
  - weights streamed as 0.5-2MB chunks through an 8-buffer SBUF ring
  - the final layer flips to batch-on-partitions (N=512 moving operand)
    so the (128, E) fp32 row block DMAs out contiguously; the host
    concatenates, adds the bias constant and broadcasts over P

walrus in this environment accepts only ONE semaphore wait per
instruction; `_legalize_waits` splits any extra waits into standalone
single-wait NoOps spliced immediately before the instruction on the same
engine stream (FIFO dispatch makes this exactly equivalent).
"""

import os
from contextlib import ExitStack

import numpy as np
import ml_dtypes

import concourse.bass as bass
import concourse.mybir as mybir
import concourse.tile as tile
from concourse.bass import _add_dep_helper
from concourse.bass_utils import run_bass_kernel_spmd

B, CLIP, P, E, H = 1024, 1024, 32, 2048, 16
NCORES = 8
BC = B // NCORES  # batch rows per core


def _build_kernel(tc, out_ap, xT, wmT, wvT, womT, woT):
    nc = tc.nc
    f32 = mybir.dt.float32
    bf16 = mybir.dt.bfloat16

    with ExitStack() as ctx:
        const_pool = ctx.enter_context(tc.tile_pool(name="const", bufs=1))
        wpool = ctx.enter_context(tc.tile_pool(name="wchunk", bufs=1))
        act_pool = ctx.enter_context(tc.tile_pool(name="act", bufs=8))
        out_pool = ctx.enter_context(tc.tile_pool(name="out", bufs=1))
        acc_pool = ctx.enter_context(
            tc.tile_pool(name="acc", bufs=8, space=bass.MemorySpace.PSUM)
        )

        # xT: (CLIP, BC) -> 8 stacked [128, 128] tiles in one DMA, on the SP
        # HWDGE queue so the Pool queue starts streaming weights immediately
        x_t = const_pool.tile([128, (CLIP // 128) * BC], bf16, name="xT_sb", tag="xT")
        nc.sync.dma_start(x_t[:], xT.rearrange("(t p) c -> p t c", p=128))
        actT = [x_t[:, bass.ts(k, BC)] for k in range(CLIP // 128)]

        # 8 statically-allocated weight ring buffers (16MB)
        NWBUF = 8
        wtiles = [
            wpool.tile([128, 4 * E], bf16, name=f"wbuf{i}", tag=f"wbuf{i}")
            for i in range(NWBUF)
        ]

        # bank-interleaved m order so consecutive matmuls hit different
        # PSUM banks (instruction-level parallelism across banks)
        m_order = [c + 4 * g for g in range(4) for c in range(4)]

        # layer 1 streams in single 512KB slabs so the first matmul can
        # start ~4us after the queue opens instead of waiting for 2MB
        layers = [
            (wmT, CLIP // 128, 1),
            (wvT, E // 128, 4),
            (womT, E // 128, 4),
            (woT, E // 128, 4),
        ]
        wdma_count = 0
        for li, (wT, nk, T) in enumerate(layers):
            last = li == len(layers) - 1
            # weight chunks: T k-slabs (T*128 rows x E cols) per DMA
            wT_r = wT.rearrange("(j t p) c -> j p t c", p=128, t=T)
            nj = nk // T
            # 16 accumulators [128,128] packed 4-per-bank into 4 PSUM tiles
            accs = [
                acc_pool.tile([128, 512], f32, name="acc", tag="acc")
                for _ in range(4)
            ]
            bank_start_mm = {}
            if last:
                out_sb = out_pool.tile([128, E], f32, name="out_sb", tag="out_sb")
                groups = None
            else:
                out_sb = None
                groups = [
                    act_pool.tile([128, 512], bf16, name="actg", tag="actg")
                    for _ in range(4)
                ]
            for j in range(nj):
                wchunk = wtiles[wdma_count % NWBUF]
                nc.gpsimd.dma_start(wchunk[:, : T * E], wT_r[j])
                wdma_count += 1
                for t in range(T):
                    k = j * T + t
                    fin = k == nk - 1
                    if last:
                        # Final layer: output orientation is free (the host
                        # reassembles), so flip to batch-on-partitions with
                        # the activation tile stationary and weight rows as
                        # a 512-wide moving operand: 64 N=512 matmuls and
                        # only 16 weight loads instead of 256 of each.
                        for c in range(4):
                            nc.tensor.matmul(
                                accs[c][:],
                                actT[k],
                                wchunk[:, t * E + c * 512 : t * E + (c + 1) * 512],
                                start=(k == 0),
                                stop=fin,
                            )
                            if fin:
                                # evacuate + store this 512-col slice while
                                # the remaining banks are still accumulating
                                nc.scalar.copy(
                                    out_sb[:, bass.ts(c, 512)], accs[c][:]
                                )
                                (nc.sync if c % 2 == 0 else nc.gpsimd).dma_start(
                                    out_ap[:, bass.ts(c, 512)],
                                    out_sb[:, bass.ts(c, 512)],
                                )
                        continue
                    # on the final k-slab go bank-major so each bank's
                    # evacuation can start while other banks still accumulate
                    order = list(range(16)) if fin else m_order
                    for m in order:
                        sl, bank = m % 4, m // 4
                        # start=True clears has_written for the WHOLE bank,
                        # so only the first slice written into each bank may
                        # set it; later slices' first matmuls overwrite via
                        # the cleared bits (and must be ordered after the
                        # clearing matmul).
                        mm = nc.tensor.matmul(
                            accs[bank][:, sl * 128 : (sl + 1) * 128],
                            wchunk[:, t * E + m * 128 : t * E + (m + 1) * 128],
                            actT[k],
                            start=(k == 0 and sl == 0),
                            stop=(fin and sl == 3),
                            skip_group_check=True,
                        )
                        if k == 0:
                            if sl == 0:
                                bank_start_mm[bank] = mm
                            else:
                                _add_dep_helper(
                                    mm.ins, bank_start_mm[bank].ins,
                                    sync=False, reason="bank clear order",
                                )
                        if fin and sl == 3:
                            nc.scalar.copy(groups[bank][:], accs[bank][:])
            if not last:
                actT = [
                    groups[k // 4][:, (k % 4) * 128 : (k % 4 + 1) * 128]
                    for k in range(E // 128)
                ]


def _legalize_waits(nc):
    """walrus here accepts only one semaphore wait per instruction.  Split
    any extra waits into standalone single-wait NoOps spliced immediately
    before the instruction on the same engine stream; engine dispatch is
    strictly FIFO, so the semantics are identical."""
    wid = [0]
    for f in nc.m.functions:
        for blk in f.blocks:
            insts = list(blk.instructions)
            new = []
            changed = False
            for inst in insts:
                si = getattr(inst, "sync_info", None)
                w = list(si.on_wait) if si is not None and si.on_wait else []
                if len(w) > 1:
                    changed = True
                    for x in w[:-1]:
                        nop = mybir.InstNoOp(
                            name=f"Wsplit-{wid[0]}", ins=[], outs=[]
                        )
                        wid[0] += 1
                        nop.engine = inst.engine
                        nop.sync_info = mybir.SyncInfo(
                            on_wait=[x], on_update=[]
                        )
                        new.append(nop)
                    upd = list(si.on_update) if si.on_update else []
                    inst.sync_info = mybir.SyncInfo(on_wait=[w[-1:][0]], on_update=upd)
                new.append(inst)
            if changed:
                blk.instructions = new


_NC_CACHE = None


def _get_nc(legalize=True):
    global _NC_CACHE
    if legalize and _NC_CACHE is not None:
        return _NC_CACHE
    nc = bass.Bass("TRN2", target_bir_lowering=False, debug=False)
    bf16 = mybir.dt.bfloat16
    xT = nc.dram_tensor("xT", (CLIP, BC), bf16, kind="ExternalInput")
    wmT = nc.dram_tensor("wmT", (CLIP, E), bf16, kind="ExternalInput")
    wvT = nc.dram_tensor("wvT", (E, E), bf16, kind="ExternalInput")
    womT = nc.dram_tensor("womT", (E, E), bf16, kind="ExternalInput")
    woT = nc.dram_tensor("woT", (E, E), bf16, kind="ExternalInput")
    out = nc.dram_tensor("out", (BC, E), mybir.dt.float32, kind="ExternalOutput")
    with tile.TileContext(nc) as tc:
        _build_kernel(
            tc,
            out.ap(),
            xT.ap(),
            wmT.ap(),
            wvT.ap(),
            womT.ap(),
            woT.ap(),
        )
    if not legalize:
        return nc
    _legalize_waits(nc)
    _NC_CACHE = nc
    return nc


LAST_RESULTS = None  # BassKernelResults of the most recent run (for profiling)


def _ensure_ntff_hook():
    """Register the axon NTFF profiling hook if the image's antenv lacks it."""
    try:
        from antenv.axon_hooks import get_axon_ntff_profile_hook  # noqa: F401

        return
    except ImportError:
        pass
    import sys as _sys
    import types as _types

    try:
        from trn_agent_boot.trn_boot import _ntff_profile_via_ctypes

        hook = _ntff_profile_via_ctypes("/opt/axon/libaxon_pjrt.so")
    except Exception:
        hook = None
    mod = _types.ModuleType("antenv.axon_hooks")
    mod._hook = hook
    mod.get_axon_ntff_profile_hook = lambda: mod._hook
    mod.set_axon_ntff_profile_hook = lambda h: setattr(mod, "_hook", h)
    _sys.modules["antenv.axon_hooks"] = mod
    import antenv

    antenv.axon_hooks = mod
    # artifact upload needs S3 egress which this sandbox doesn't have
    import concourse.bass_utils as _bu

    _bu.upload_artifacts = lambda tmpdir: tmpdir


def kernel(image_embs, Wm, bm, prefix_queries, Win, bin, Wo_mha, bo_mha, Wo, bo):
    X = np.asarray(image_embs, dtype=np.float32)
    Wm = np.asarray(Wm, dtype=np.float32)
    bm = np.asarray(bm, dtype=np.float32)
    Win = np.asarray(Win, dtype=np.float32)
    bin_ = np.asarray(bin, dtype=np.float32)
    Wo_mha = np.asarray(Wo_mha, dtype=np.float32)
    bo_mha = np.asarray(bo_mha, dtype=np.float32)
    Wo = np.asarray(Wo, dtype=np.float32)
    bo = np.asarray(bo, dtype=np.float32)

    Wv = Win[2 * E : 3 * E]
    bv = bin_[2 * E : 3 * E]

    # batch-independent bias contribution (exact, fp32 on host)
    c = ((bm @ Wv.T + bv) @ Wo_mha.T + bo_mha) @ Wo.T + bo  # (E,)

    bf = ml_dtypes.bfloat16
    shared = {
        "wmT": np.ascontiguousarray(Wm.T).astype(bf),
        "wvT": np.ascontiguousarray(Wv.T).astype(bf),
        "womT": np.ascontiguousarray(Wo_mha.T).astype(bf),
        "woT": np.ascontiguousarray(Wo.T).astype(bf),
    }
    in_maps = []
    for ci in range(NCORES):
        xs = X[ci * BC : (ci + 1) * BC]  # (BC, CLIP)
        m = dict(shared)
        m["xT"] = np.ascontiguousarray(xs.T).astype(bf)
        in_maps.append(m)

    nc = _get_nc()
    trace = bool(int(os.environ.get("KERNEL_TRACE", "0")))
    if trace:
        _ensure_ntff_hook()
    res = run_bass_kernel_spmd(
        nc, in_maps, core_ids=list(range(NCORES)), trace=trace
    )
    global LAST_RESULTS
    LAST_RESULTS = res

    rows = np.concatenate(
        [np.asarray(res.results[ci]["out"]) for ci in range(NCORES)], axis=0
    )  # (B, E) float32
    rows = rows + c[None, :].astype(np.float32)
    return np.broadcast_to(rows[:, None, :], (B, P, E))



# revision 2
# speedup vs baseline: 1.1775x; 1.1775x over previous
"""Trainium2 Bass kernel for nn_CrossAttentionAdapter.

Math note: the reference's attention has kv_len == 1, so the softmax over a
length-1 axis is exactly 1.0 and the attention output is just `v` broadcast
over the P=32 prefix positions.  The whole module therefore collapses to a
chain of 4 matmuls applied to image_embs:

    row = image_embs @ Wm.T @ Wv.T @ Wo_mha.T @ Wo.T  (+ bias constant)
    out[b, p, :] = row[b, :]          for every p in range(32)

where Wv = Win[2E:3E].  The bias terms contribute a batch-independent
constant vector c = ((bm @ Wv.T + bv) @ Wo_mha.T + bo_mha) @ Wo.T + bo,
added on the host.  prefix_queries / Wq / Wk never affect the output.

Device strategy (pure data parallel, 8 cores), v2:
  - batch (1024) sharded 8-ways -> 128 rows per core
  - the three big weights (Wv, Wo_mha, Wo) are quantized on the host to
    int8 with per-input-channel (per-k) scales; Wm and all activations
    are fp16.  This halves the dominant weight HBM traffic (28MB ->
    ~17MB per core), which was the baseline bottleneck (~76us DMA busy).
  - int8 chunks are upcast to fp16 on-chip: a plain dtype-cast copy,
    split DVE (3 k-slabs, 2 elem/cycle/lane) / ACT (1 k-slab); the
    dequant scales are NOT applied here -- per-k scales commute through
    the matmul onto the activations, so each layer's input scales are
    applied for free in the PREVIOUS layer's PSUM-evacuation copy
    (activation Copy with a per-partition scale vector).
  - all 4 layers run weight-stationary (weight tile lhsT [k,m], acts
    moving N=128): trace shows LDWEIGHTS is ~fully hidden at this shape
    (56-58ns/MM steady state).  Layer outputs stay feature-on-partitions
    so the chain needs no transposes; the host untransposes the final
    (feat, batch) tiles during unshard.
  - 16 output-tile accumulators per layer packed 4-per-PSUM-bank as
    [128,512] tiles; bank-interleaved m order for ILP across banks.

walrus in this environment accepts only ONE semaphore wait per
instruction; `_legalize_waits` splits any extra waits into standalone
single-wait NoOps spliced immediately before the instruction on the same
engine stream (FIFO dispatch makes this exactly equivalent).
"""

import os
from contextlib import ExitStack

import numpy as np
import ml_dtypes

import concourse.bass as bass
import concourse.mybir as mybir
import concourse.tile as tile
from concourse.bass import _add_dep_helper
from concourse.bass_utils import run_bass_kernel_spmd

B, CLIP, P, E, H = 1024, 1024, 32, 2048, 16
NCORES = 8
BC = B // NCORES  # batch rows per core

# dequant split: DVE casts the first DVE_COLS of each int8 chunk, ACT the rest
CHUNK_COLS = 4 * E          # 4 k-slabs per int8 chunk
DVE_COLS = 3 * E            # cols 0:6144 on DVE, rest on ACT


def _build_kernel(tc, out_ap, xT, wmT, wq_aps, s_aps):
    nc = tc.nc
    f32 = mybir.dt.float32
    f16 = mybir.dt.float16
    i8 = mybir.dt.int8
    COPY = mybir.ActivationFunctionType.Copy

    with ExitStack() as ctx:
        const_pool = ctx.enter_context(tc.tile_pool(name="const", bufs=1))
        i8pool = ctx.enter_context(tc.tile_pool(name="i8chunk", bufs=1))
        wpool = ctx.enter_context(tc.tile_pool(name="wchunk", bufs=1))
        act_pool = ctx.enter_context(tc.tile_pool(name="act", bufs=8))
        out_pool = ctx.enter_context(tc.tile_pool(name="out", bufs=1))
        acc_pool = ctx.enter_context(
            tc.tile_pool(name="acc", bufs=8, space=bass.MemorySpace.PSUM)
        )

        # xT: (CLIP, BC) -> 8 stacked [128, 128] tiles in one DMA on the SP
        # HWDGE queue so the Pool queue starts streaming weights immediately
        x_t = const_pool.tile([128, (CLIP // 128) * BC], f16, name="xT_sb", tag="xT")
        nc.sync.dma_start(x_t[:], xT.rearrange("(t p) c -> p t c", p=128))
        actT = [x_t[:, bass.ts(k, BC)] for k in range(CLIP // 128)]

        # per-layer input-scale tiles for layers 2..4: [128, 16] fp32,
        # column mt = scales for features mt*128+p of the previous output
        s_sb = []
        for li, s_ap in enumerate(s_aps):
            st = const_pool.tile([128, 16], f32, name=f"s{li+2}_sb", tag=f"s{li+2}")
            nc.sync.dma_start(st[:], s_ap)
            s_sb.append(st)

        # rings: int8 chunks (1MB) and fp16 weight chunks (2MB)
        NI8 = 4
        i8tiles = [
            i8pool.tile([128, CHUNK_COLS], i8, name=f"i8buf{i}", tag=f"i8buf{i}")
            for i in range(NI8)
        ]
        NWBUF = 4
        wtiles = [
            wpool.tile([128, CHUNK_COLS], f16, name=f"wbuf{i}", tag=f"wbuf{i}")
            for i in range(NWBUF)
        ]

        # bank-interleaved m order so consecutive matmuls hit different
        # PSUM banks (instruction-level parallelism across banks)
        m_order = [c + 4 * g for g in range(4) for c in range(4)]

        # layers: (weight_ap, n_k_slabs, slabs_per_chunk, is_int8)
        # L1 fp16 streams 1MB chunks (T=2) so the first matmul starts early
        layers = [
            (wmT, CLIP // 128, 2, False),
            (wq_aps[0], E // 128, 4, True),
            (wq_aps[1], E // 128, 4, True),
            (wq_aps[2], E // 128, 4, True),
        ]
        i8_count = 0
        w_count = 0
        for li, (wT, nk, T, quant) in enumerate(layers):
            last = li == len(layers) - 1
            wT_r = wT.rearrange("(j t p) c -> j p t c", p=128, t=T)
            nj = nk // T
            accs = [
                acc_pool.tile([128, 512], f32, name="acc", tag="acc")
                for _ in range(4)
            ]
            bank_start_mm = {}
            if last:
                out_sb = out_pool.tile([128, E], f32, name="out_sb", tag="out_sb")
                groups = None
            else:
                out_sb = None
                groups = [
                    act_pool.tile([128, 512], f16, name="actg", tag="actg")
                    for _ in range(4)
                ]
                snext = s_sb[li]
            for j in range(nj):
                if quant:
                    ichunk = i8tiles[i8_count % NI8]
                    i8_count += 1
                    nc.gpsimd.dma_start(ichunk[:], wT_r[j])
                    wchunk = wtiles[w_count % NWBUF]
                    w_count += 1
                    # dequant: plain dtype cast, split DVE / ACT
                    nc.vector.tensor_copy(
                        wchunk[:, :DVE_COLS], ichunk[:, :DVE_COLS]
                    )
                    nc.scalar.copy(
                        wchunk[:, DVE_COLS:], ichunk[:, DVE_COLS:]
                    )
                else:
                    wchunk = wtiles[w_count % NWBUF]
                    w_count += 1
                    nc.gpsimd.dma_start(wchunk[:, : T * E], wT_r[j])
                for t in range(T):
                    k = j * T + t
                    fin = k == nk - 1
                    # on the final k-slab go bank-major so each bank's
                    # evacuation can start while other banks still accumulate
                    order = list(range(16)) if fin else m_order
                    for m in order:
                        sl, bank = m % 4, m // 4
                        # start=True clears has_written for the WHOLE bank,
                        # so only the first slice written into each bank may
                        # set it; later slices' first matmuls overwrite via
                        # the cleared bits (and must be ordered after the
                        # clearing matmul).
                        mm = nc.tensor.matmul(
                            accs[bank][:, sl * 128 : (sl + 1) * 128],
                            wchunk[:, t * E + m * 128 : t * E + (m + 1) * 128],
                            actT[k],
                            start=(k == 0 and sl == 0),
                            stop=(fin and sl == 3),
                            skip_group_check=True,
                        )
                        if k == 0:
                            if sl == 0:
                                bank_start_mm[bank] = mm
                            else:
                                _add_dep_helper(
                                    mm.ins, bank_start_mm[bank].ins,
                                    sync=False, reason="bank clear order",
                                )
                        if fin and sl == 3:
                            if last:
                                # plain fp32 evacuation of m-tiles 4b..4b+3,
                                # then store while other banks still run
                                nc.scalar.copy(
                                    out_sb[:, bank * 512 : (bank + 1) * 512],
                                    accs[bank][:],
                                )
                                (nc.sync if bank % 2 == 0 else nc.gpsimd).dma_start(
                                    out_ap[:, bass.ts(bank, 512)],
                                    out_sb[:, bass.ts(bank, 512)],
                                )
                            else:
                                # evacuate with the NEXT layer's per-k input
                                # scales (per-partition scale vector)
                                for sl2 in range(4):
                                    mt = bank * 4 + sl2
                                    nc.scalar.activation(
                                        groups[bank][:, sl2 * 128 : (sl2 + 1) * 128],
                                        accs[bank][:, sl2 * 128 : (sl2 + 1) * 128],
                                        COPY,
                                        scale=snext[:, mt : mt + 1],
                                    )
            if not last:
                actT = [
                    groups[k // 4][:, (k % 4) * 128 : (k % 4 + 1) * 128]
                    for k in range(E // 128)
                ]


def _legalize_waits(nc):
    """walrus here accepts only one semaphore wait per instruction.  Split
    any extra waits into standalone single-wait NoOps spliced immediately
    before the instruction on the same engine stream; engine dispatch is
    strictly FIFO, so the semantics are identical."""
    wid = [0]
    for f in nc.m.functions:
        for blk in f.blocks:
            insts = list(blk.instructions)
            new = []
            changed = False
            for inst in insts:
                si = getattr(inst, "sync_info", None)
                w = list(si.on_wait) if si is not None and si.on_wait else []
                if len(w) > 1:
                    changed = True
                    for x in w[:-1]:
                        nop = mybir.InstNoOp(
                            name=f"Wsplit-{wid[0]}", ins=[], outs=[]
                        )
                        wid[0] += 1
                        nop.engine = inst.engine
                        nop.sync_info = mybir.SyncInfo(
                            on_wait=[x], on_update=[]
                        )
                        new.append(nop)
                    upd = list(si.on_update) if si.on_update else []
                    inst.sync_info = mybir.SyncInfo(on_wait=[w[-1:][0]], on_update=upd)
                new.append(inst)
            if changed:
                blk.instructions = new


_NC_CACHE = None


def _get_nc(legalize=True):
    global _NC_CACHE
    if legalize and _NC_CACHE is not None:
        return _NC_CACHE
    nc = bass.Bass("TRN2", target_bir_lowering=False, debug=False)
    f16 = mybir.dt.float16
    i8 = mybir.dt.int8
    f32 = mybir.dt.float32
    xT = nc.dram_tensor("xT", (CLIP, BC), f16, kind="ExternalInput")
    wmT = nc.dram_tensor("wmT", (CLIP, E), f16, kind="ExternalInput")
    wvq = nc.dram_tensor("wvq", (E, E), i8, kind="ExternalInput")
    womq = nc.dram_tensor("womq", (E, E), i8, kind="ExternalInput")
    woq = nc.dram_tensor("woq", (E, E), i8, kind="ExternalInput")
    s2 = nc.dram_tensor("s2", (128, 16), f32, kind="ExternalInput")
    s3 = nc.dram_tensor("s3", (128, 16), f32, kind="ExternalInput")
    s4 = nc.dram_tensor("s4", (128, 16), f32, kind="ExternalInput")
    # out is the TRANSPOSED row block: out[p, mt*128 + b] = y[mt*128+p, b]
    out = nc.dram_tensor("out", (128, E), f32, kind="ExternalOutput")
    with tile.TileContext(nc) as tc:
        _build_kernel(
            tc,
            out.ap(),
            xT.ap(),
            wmT.ap(),
            [wvq.ap(), womq.ap(), woq.ap()],
            [s2.ap(), s3.ap(), s4.ap()],
        )
    if not legalize:
        return nc
    _legalize_waits(nc)
    _NC_CACHE = nc
    return nc


LAST_RESULTS = None  # BassKernelResults of the most recent run (for profiling)


def _ensure_ntff_hook():
    """Register the axon NTFF profiling hook if the image's antenv lacks it."""
    try:
        from antenv.axon_hooks import get_axon_ntff_profile_hook  # noqa: F401

        return
    except ImportError:
        pass
    import sys as _sys
    import types as _types

    try:
        from trn_agent_boot.trn_boot import _ntff_profile_via_ctypes

        hook = _ntff_profile_via_ctypes("/opt/axon/libaxon_pjrt.so")
    except Exception:
        hook = None
    mod = _types.ModuleType("antenv.axon_hooks")
    mod._hook = hook
    mod.get_axon_ntff_profile_hook = lambda: mod._hook
    mod.set_axon_ntff_profile_hook = lambda h: setattr(mod, "_hook", h)
    _sys.modules["antenv.axon_hooks"] = mod
    import antenv

    antenv.axon_hooks = mod
    # artifact upload needs S3 egress which this sandbox doesn't have
    import concourse.bass_utils as _bu

    _bu.upload_artifacts = lambda tmpdir: tmpdir


def _quant_per_k(W):
    """Per-input-channel int8 quantization of W.T: returns (Q (K,M) int8,
    s (K,) fp32) with W.T = s[:,None] * Q exactly at the max magnitude."""
    wT = np.ascontiguousarray(W.T).astype(np.float32)
    s = np.abs(wT).max(axis=1) / 127.0
    s = np.where(s == 0, 1.0, s)
    Q = np.rint(wT / s[:, None]).astype(np.int8)
    return Q, s.astype(np.float32)


def kernel(image_embs, Wm, bm, prefix_queries, Win, bin, Wo_mha, bo_mha, Wo, bo):
    X = np.asarray(image_embs, dtype=np.float32)
    Wm = np.asarray(Wm, dtype=np.float32)
    bm = np.asarray(bm, dtype=np.float32)
    Win = np.asarray(Win, dtype=np.float32)
    bin_ = np.asarray(bin, dtype=np.float32)
    Wo_mha = np.asarray(Wo_mha, dtype=np.float32)
    bo_mha = np.asarray(bo_mha, dtype=np.float32)
    Wo = np.asarray(Wo, dtype=np.float32)
    bo = np.asarray(bo, dtype=np.float32)

    Wv = Win[2 * E : 3 * E]
    bv = bin_[2 * E : 3 * E]

    # batch-independent bias contribution (exact, fp32 on host)
    c = ((bm @ Wv.T + bv) @ Wo_mha.T + bo_mha) @ Wo.T + bo  # (E,)

    qv, sv = _quant_per_k(Wv)       # scales of L2 inputs -> applied at L1 evac
    qom, som = _quant_per_k(Wo_mha)  # L3 inputs -> L2 evac
    qo, so = _quant_per_k(Wo)       # L4 inputs -> L3 evac

    shared = {
        "wmT": np.ascontiguousarray(Wm.T).astype(np.float16),
        "wvq": qv,
        "womq": qom,
        "woq": qo,
        "s2": np.ascontiguousarray(sv.reshape(16, 128).T),
        "s3": np.ascontiguousarray(som.reshape(16, 128).T),
        "s4": np.ascontiguousarray(so.reshape(16, 128).T),
    }
    in_maps = []
    for ci in range(NCORES):
        xs = X[ci * BC : (ci + 1) * BC]  # (BC, CLIP)
        m = dict(shared)
        m["xT"] = np.ascontiguousarray(xs.T).astype(np.float16)
        in_maps.append(m)

    nc = _get_nc()
    trace = bool(int(os.environ.get("KERNEL_TRACE", "0")))
    if trace:
        _ensure_ntff_hook()
    res = run_bass_kernel_spmd(
        nc, in_maps, core_ids=list(range(NCORES)), trace=trace
    )
    global LAST_RESULTS
    LAST_RESULTS = res

    # out[p, mt*128+b] = y[mt*128+p, b]; untranspose per 128-col tile
    rows = np.empty((B, E), np.float32)
    for ci in range(NCORES):
        o = np.asarray(res.results[ci]["out"]).reshape(128, 16, BC)
        rows[ci * BC : (ci + 1) * BC] = o.transpose(2, 1, 0).reshape(BC, E)
    rows = rows + c[None, :].astype(np.float32)
    return np.broadcast_to(rows[:, None, :], (B, P, E))


# revision 4
# speedup vs baseline: 1.3303x; 1.1297x over previous
"""Trainium2 Bass kernel for nn_CrossAttentionAdapter.

Math note: the reference's attention has kv_len == 1, so the softmax over a
length-1 axis is exactly 1.0 and the attention output is just `v` broadcast
over the P=32 prefix positions.  The whole module therefore collapses to a
chain of 4 matmuls applied to image_embs:

    row = image_embs @ Wm.T @ Wv.T @ Wo_mha.T @ Wo.T  (+ bias constant)
    out[b, p, :] = row[b, :]          for every p in range(32)

where Wv = Win[2E:3E].  The bias terms contribute a batch-independent
constant vector c = ((bm @ Wv.T + bv) @ Wo_mha.T + bo_mha) @ Wo.T + bo,
added on the host.  prefix_queries / Wq / Wk never affect the output.

Device strategy (pure data parallel, 8 cores), v3:
  - batch (1024) sharded 8-ways -> 128 rows per core
  - ALL four weights are quantized on the host to int8 with
    per-input-channel (per-k) scales; activations are fp16.  This halves
    the weight HBM traffic (28MB -> 14MB per core), which was the
    baseline bottleneck (~76us DMA busy at 102us total).
  - int8 chunks are upcast to fp16 on-chip: a plain dtype-cast copy,
    split DVE (2 elem/cycle/lane) / ACT; the dequant scales are NOT
    applied here -- per-k scales commute through the matmul onto the
    activations, so layer l's input scales are applied for free in layer
    l-1's PSUM-evacuation copy (activation Copy with a per-partition
    scale vector), and layer 1's scales are folded into xT on the host.
  - all 4 layers run weight-stationary (weight tile lhsT [k,m], acts
    moving N=128): LDWEIGHTS is ~fully hidden at this shape (61ns/MM
    measured).  Layer outputs stay feature-on-partitions so the chain
    needs no transposes; the host untransposes the final (feat, batch)
    tiles during unshard.
  - 16 output-tile accumulators per layer packed 4-per-PSUM-bank as
    [128,512] tiles; bank-interleaved m order for ILP across banks.
  - chunk plans: small leading chunks (L1: 1,1,2,2,2 slabs) so the first
    matmul starts ~9us, and small trailing chunks (L4: ...,2,2) to cut
    the DMA->dequant->PE tail latency.

walrus in this environment accepts only ONE semaphore wait per
instruction; `_legalize_waits` splits any extra waits into standalone
single-wait NoOps spliced immediately before the instruction on the same
engine stream (FIFO dispatch makes this exactly equivalent).
"""

import os
from contextlib import ExitStack

import numpy as np

import concourse.bass as bass
import concourse.mybir as mybir
import concourse.tile as tile
from concourse.bass import _add_dep_helper
from concourse.bass_utils import run_bass_kernel_spmd

B, CLIP, P, E, H = 1024, 1024, 32, 2048, 16
NCORES = 8
BC = B // NCORES  # batch rows per core

CHUNK_COLS = 4 * E          # largest chunk: 4 k-slabs
DVE_FRAC = 47               # DVE's share of dequant cols, out of 64


def _dve_cols(total):
    return ((DVE_FRAC * total) // 64) // 128 * 128


# per-layer k-slab chunk plans (sum == n_k_slabs)
PLANS = [
    [1, 1, 2, 2, 2],        # L1: 8 slabs,  fast start
    [4, 4, 4, 4],           # L2
    [4, 4, 4, 4],           # L3
    [4, 4, 4, 2, 2],        # L4: short tail
]


def _build_kernel(tc, out_ap, xT, wq_aps, s_aps):
    nc = tc.nc
    f32 = mybir.dt.float32
    f16 = mybir.dt.float16
    i8 = mybir.dt.int8
    COPY = mybir.ActivationFunctionType.Copy

    with ExitStack() as ctx:
        const_pool = ctx.enter_context(tc.tile_pool(name="const", bufs=1))
        i8pool = ctx.enter_context(tc.tile_pool(name="i8chunk", bufs=1))
        wpool = ctx.enter_context(tc.tile_pool(name="wchunk", bufs=1))
        act_pool = ctx.enter_context(tc.tile_pool(name="act", bufs=8))
        out_pool = ctx.enter_context(tc.tile_pool(name="out", bufs=1))
        acc_pool = ctx.enter_context(
            tc.tile_pool(name="acc", bufs=8, space=bass.MemorySpace.PSUM)
        )

        # xT: (CLIP, BC) -> 8 stacked [128, 128] tiles in one DMA on the SP
        # HWDGE queue so the Pool queue starts streaming weights immediately
        x_t = const_pool.tile([128, (CLIP // 128) * BC], f16, name="xT_sb", tag="xT")
        nc.sync.dma_start(x_t[:], xT.rearrange("(t p) c -> p t c", p=128))
        actT = [x_t[:, bass.ts(k, BC)] for k in range(CLIP // 128)]

        # per-layer input-scale tiles for layers 2..4: [128, 16] fp32,
        # column mt = scales for features mt*128+p of the previous output
        s_sb = []
        for li, s_ap in enumerate(s_aps):
            st = const_pool.tile([128, 16], f32, name=f"s{li+2}_sb", tag=f"s{li+2}")
            nc.sync.dma_start(st[:], s_ap)
            s_sb.append(st)

        NI8 = 6
        i8tiles = [
            i8pool.tile([128, CHUNK_COLS], i8, name=f"i8buf{i}", tag=f"i8buf{i}")
            for i in range(NI8)
        ]
        NWBUF = 5
        wtiles = [
            wpool.tile([128, CHUNK_COLS], f16, name=f"wbuf{i}", tag=f"wbuf{i}")
            for i in range(NWBUF)
        ]

        # bank-interleaved m order so consecutive matmuls hit different
        # PSUM banks (instruction-level parallelism across banks)
        m_order = [c + 4 * g for g in range(4) for c in range(4)]

        i8_count = 0
        w_count = 0
        for li, wT in enumerate(wq_aps):
            plan = PLANS[li]
            nk = sum(plan)
            last = li == len(wq_aps) - 1
            # slab-major stride view: [128, nk, E]
            wT_v = wT.rearrange("(s p) c -> p s c", p=128)
            accs = [
                acc_pool.tile([128, 512], f32, name="acc", tag="acc")
                for _ in range(4)
            ]
            bank_start_mm = {}
            if last:
                out_sb = out_pool.tile([128, E], f32, name="out_sb", tag="out_sb")
                groups = None
            else:
                out_sb = None
                groups = [
                    act_pool.tile([128, 512], f16, name="actg", tag="actg")
                    for _ in range(4)
                ]
                snext = s_sb[li]
            s0 = 0
            for T in plan:
                cols = T * E
                dcols = _dve_cols(cols)
                ichunk = i8tiles[i8_count % NI8]
                i8_count += 1
                nc.gpsimd.dma_start(ichunk[:, :cols], wT_v[:, s0 : s0 + T])
                wchunk = wtiles[w_count % NWBUF]
                w_count += 1
                # dequant: plain dtype cast, split DVE / ACT
                nc.vector.tensor_copy(wchunk[:, :dcols], ichunk[:, :dcols])
                nc.scalar.copy(wchunk[:, dcols:cols], ichunk[:, dcols:cols])
                for t in range(T):
                    k = s0 + t
                    fin = k == nk - 1
                    # on the final k-slab go bank-major so each bank's
                    # evacuation can start while other banks still accumulate
                    order = list(range(16)) if fin else m_order
                    for m in order:
                        sl, bank = m % 4, m // 4
                        # start=True clears has_written for the WHOLE bank,
                        # so only the first slice written into each bank may
                        # set it; later slices' first matmuls overwrite via
                        # the cleared bits (and must be ordered after the
                        # clearing matmul).
                        mm = nc.tensor.matmul(
                            accs[bank][:, sl * 128 : (sl + 1) * 128],
                            wchunk[:, t * E + m * 128 : t * E + (m + 1) * 128],
                            actT[k],
                            start=(k == 0 and sl == 0),
                            stop=(fin and sl == 3),
                            skip_group_check=True,
                        )
                        if k == 0:
                            if sl == 0:
                                bank_start_mm[bank] = mm
                            else:
                                _add_dep_helper(
                                    mm.ins, bank_start_mm[bank].ins,
                                    sync=False, reason="bank clear order",
                                )
                        if fin and sl == 3:
                            if last:
                                # plain fp32 evacuation of m-tiles 4b..4b+3,
                                # then store while other banks still run
                                nc.scalar.copy(
                                    out_sb[:, bank * 512 : (bank + 1) * 512],
                                    accs[bank][:],
                                )
                                (nc.sync if bank % 2 == 0 else nc.gpsimd).dma_start(
                                    out_ap[:, bass.ts(bank, 512)],
                                    out_sb[:, bass.ts(bank, 512)],
                                )
                            else:
                                # evacuate with the NEXT layer's per-k input
                                # scales (per-partition scale vector)
                                for sl2 in range(4):
                                    mt = bank * 4 + sl2
                                    nc.scalar.activation(
                                        groups[bank][:, sl2 * 128 : (sl2 + 1) * 128],
                                        accs[bank][:, sl2 * 128 : (sl2 + 1) * 128],
                                        COPY,
                                        scale=snext[:, mt : mt + 1],
                                    )
                s0 += T
            if not last:
                actT = [
                    groups[k // 4][:, (k % 4) * 128 : (k % 4 + 1) * 128]
                    for k in range(E // 128)
                ]


def _legalize_waits(nc):
    """walrus here accepts only one semaphore wait per instruction.  Split
    any extra waits into standalone single-wait NoOps spliced immediately
    before the instruction on the same engine stream; engine dispatch is
    strictly FIFO, so the semantics are identical."""
    wid = [0]
    for f in nc.m.functions:
        for blk in f.blocks:
            insts = list(blk.instructions)
            new = []
            changed = False
            for inst in insts:
                si = getattr(inst, "sync_info", None)
                w = list(si.on_wait) if si is not None and si.on_wait else []
                if len(w) > 1:
                    changed = True
                    for x in w[:-1]:
                        nop = mybir.InstNoOp(
                            name=f"Wsplit-{wid[0]}", ins=[], outs=[]
                        )
                        wid[0] += 1
                        nop.engine = inst.engine
                        nop.sync_info = mybir.SyncInfo(
                            on_wait=[x], on_update=[]
                        )
                        new.append(nop)
                    upd = list(si.on_update) if si.on_update else []
                    inst.sync_info = mybir.SyncInfo(on_wait=[w[-1:][0]], on_update=upd)
                new.append(inst)
            if changed:
                blk.instructions = new


_NC_CACHE = None


def _get_nc(legalize=True):
    global _NC_CACHE
    if legalize and _NC_CACHE is not None:
        return _NC_CACHE
    nc = bass.Bass("TRN2", target_bir_lowering=False, debug=False)
    f16 = mybir.dt.float16
    i8 = mybir.dt.int8
    f32 = mybir.dt.float32
    xT = nc.dram_tensor("xT", (CLIP, BC), f16, kind="ExternalInput")
    wmq = nc.dram_tensor("wmq", (CLIP, E), i8, kind="ExternalInput")
    wvq = nc.dram_tensor("wvq", (E, E), i8, kind="ExternalInput")
    womq = nc.dram_tensor("womq", (E, E), i8, kind="ExternalInput")
    woq = nc.dram_tensor("woq", (E, E), i8, kind="ExternalInput")
    s2 = nc.dram_tensor("s2", (128, 16), f32, kind="ExternalInput")
    s3 = nc.dram_tensor("s3", (128, 16), f32, kind="ExternalInput")
    s4 = nc.dram_tensor("s4", (128, 16), f32, kind="ExternalInput")
    # out is the TRANSPOSED row block: out[p, mt*128 + b] = y[mt*128+p, b]
    out = nc.dram_tensor("out", (128, E), f32, kind="ExternalOutput")
    with tile.TileContext(nc) as tc:
        _build_kernel(
            tc,
            out.ap(),
            xT.ap(),
            [wmq.ap(), wvq.ap(), womq.ap(), woq.ap()],
            [s2.ap(), s3.ap(), s4.ap()],
        )
    if not legalize:
        return nc
    _legalize_waits(nc)
    _NC_CACHE = nc
    return nc


LAST_RESULTS = None  # BassKernelResults of the most recent run (for profiling)


def _ensure_ntff_hook():
    """Register the axon NTFF profiling hook if the image's antenv lacks it."""
    try:
        from antenv.axon_hooks import get_axon_ntff_profile_hook  # noqa: F401

        return
    except ImportError:
        pass
    import sys as _sys
    import types as _types

    try:
        from trn_agent_boot.trn_boot import _ntff_profile_via_ctypes

        hook = _ntff_profile_via_ctypes("/opt/axon/libaxon_pjrt.so")
    except Exception:
        hook = None
    mod = _types.ModuleType("antenv.axon_hooks")
    mod._hook = hook
    mod.get_axon_ntff_profile_hook = lambda: mod._hook
    mod.set_axon_ntff_profile_hook = lambda h: setattr(mod, "_hook", h)
    _sys.modules["antenv.axon_hooks"] = mod
    import antenv

    antenv.axon_hooks = mod
    # artifact upload needs S3 egress which this sandbox doesn't have
    import concourse.bass_utils as _bu

    _bu.upload_artifacts = lambda tmpdir: tmpdir


def _quant_per_k(W):
    """Per-input-channel int8 quantization of W.T: returns (Q (K,M) int8,
    s (K,) fp32) with W.T ~= s[:,None] * Q."""
    wT = np.ascontiguousarray(W.T).astype(np.float32)
    s = np.abs(wT).max(axis=1) / 127.0
    s = np.where(s == 0, 1.0, s)
    Q = np.rint(wT / s[:, None]).astype(np.int8)
    return Q, s.astype(np.float32)


def kernel(image_embs, Wm, bm, prefix_queries, Win, bin, Wo_mha, bo_mha, Wo, bo):
    X = np.asarray(image_embs, dtype=np.float32)
    Wm = np.asarray(Wm, dtype=np.float32)
    bm = np.asarray(bm, dtype=np.float32)
    Win = np.asarray(Win, dtype=np.float32)
    bin_ = np.asarray(bin, dtype=np.float32)
    Wo_mha = np.asarray(Wo_mha, dtype=np.float32)
    bo_mha = np.asarray(bo_mha, dtype=np.float32)
    Wo = np.asarray(Wo, dtype=np.float32)
    bo = np.asarray(bo, dtype=np.float32)

    Wv = Win[2 * E : 3 * E]
    bv = bin_[2 * E : 3 * E]

    # batch-independent bias contribution (exact, fp32 on host)
    c = ((bm @ Wv.T + bv) @ Wo_mha.T + bo_mha) @ Wo.T + bo  # (E,)

    qm, sm = _quant_per_k(Wm)        # L1 input scales -> folded into xT
    qv, sv = _quant_per_k(Wv)        # L2 input scales -> applied at L1 evac
    qom, som = _quant_per_k(Wo_mha)  # L3 -> L2 evac
    qo, so = _quant_per_k(Wo)        # L4 -> L3 evac

    shared = {
        "wmq": qm,
        "wvq": qv,
        "womq": qom,
        "woq": qo,
        "s2": np.ascontiguousarray(sv.reshape(16, 128).T),
        "s3": np.ascontiguousarray(som.reshape(16, 128).T),
        "s4": np.ascontiguousarray(so.reshape(16, 128).T),
    }
    in_maps = []
    for ci in range(NCORES):
        xs = X[ci * BC : (ci + 1) * BC]  # (BC, CLIP)
        m = dict(shared)
        # xT carries L1's per-k dequant scales
        m["xT"] = np.ascontiguousarray(xs.T * sm[:, None]).astype(np.float16)
        in_maps.append(m)

    nc = _get_nc()
    trace = bool(int(os.environ.get("KERNEL_TRACE", "0")))
    if trace:
        _ensure_ntff_hook()
    res = run_bass_kernel_spmd(
        nc, in_maps, core_ids=list(range(NCORES)), trace=trace
    )
    global LAST_RESULTS
    LAST_RESULTS = res

    # out[p, mt*128+b] = y[mt*128+p, b]; untranspose per 128-col tile
    rows = np.empty((B, E), np.float32)
    for ci in range(NCORES):
        o = np.asarray(res.results[ci]["out"]).reshape(128, 16, BC)
        rows[ci * BC : (ci + 1) * BC] = o.transpose(2, 1, 0).reshape(BC, E)
    rows = rows + c[None, :].astype(np.float32)
    return np.broadcast_to(rows[:, None, :], (B, P, E))
